# revision 7
# baseline (speedup 1.0000x reference)
"""Multi-head causal attention (QKV proj + attention + O proj) on 8 TRN2 cores.

Sharding: data-parallel over batch (4) x tensor-parallel over heads (2 groups
of 8 heads).  Core c handles batch c//2, head-group c%2.  Each core computes
its group's partial o_proj output; the host sums the two partials per batch.

Layout strategy (all activations arrive pre-transposed from the host, so the
kernel never transposes on-device):
  - qT, kT per head-pair M-tile: (128 head-dims, L) from  W.T-slice @ X.T
  - v natural (tokens, head-dims) with a fused ones-column for the softmax
    denominator: av_psum = v_aug.T @ P.T gives (65, 2, Nq) where row 64 is
    the per-query sum of probabilities.
  - scores are computed transposed (keys on partitions, queries free); the
    two heads of an M-tile use PE row-tiles (0,0)/(64,0) and run concurrent.
  - av accumulates over ALL key blocks of a query chunk in one PSUM bank
    pair (no SBUF accumulator).  Softmax denominators: DVE reciprocal of the
    ones-row, GpSimd partition_broadcast across the 64 head dims, DVE mult.
  - causal masking multiplies a small triangular mask after exp on diagonal
    128-blocks; fully-masked ranges are never computed.
Compute dtype bf16 (fp32 PSUM accumulation); bf16 partial outputs summed in
fp32 on the host.

Schedule: wave per query chunk qc.  Wave qc runs attention(qc) interleaved
with k/v projections of chunk qc (needed only by the diagonal blocks),
q projection of chunk qc+1, o_proj of chunk qc-1, and x-chunk prefetch, so
the PE-bound projections fill the scalar-bound (exp) attention phases and
output DMA is spread across the whole kernel instead of a tail wave.
"""

import numpy as np
import ml_dtypes

import concourse.bass as bass
import concourse.tile as tile
from concourse import bacc, mybir

D_MODEL = 1024
N_HEADS = 16
D_K = 64
B, L = 4, 2048
TP = 2                  # head groups
GD = D_MODEL // TP      # 512 head-dims per group
P = 128
NQ = 512                # query chunk (one fp32 PSUM bank)
N_MT = GD // P          # 4 M-tiles (head pairs) per group
N_KT = D_MODEL // P     # 8 contraction tiles over model dim
N_TT = L // P           # 16 token tiles
N_QC = L // NQ          # 4 query chunks
BF16 = mybir.dt.bfloat16
F32 = mybir.dt.float32
NPBF16 = ml_dtypes.bfloat16
AF = mybir.ActivationFunctionType
ALU = mybir.AluOpType


def build_nc() -> bass.Bass:
    nc = bacc.Bacc("TRN2", target_bir_lowering=False)

    xqT = nc.dram_tensor("xqT", [D_MODEL, L], BF16, kind="ExternalInput")
    xkT = nc.dram_tensor("xkT", [D_MODEL, L], BF16, kind="ExternalInput")
    xvT = nc.dram_tensor("xvT", [D_MODEL, L], BF16, kind="ExternalInput")
    wqT = nc.dram_tensor("wqT", [D_MODEL, GD], BF16, kind="ExternalInput")
    wkT = nc.dram_tensor("wkT", [D_MODEL, GD], BF16, kind="ExternalInput")
    wvT = nc.dram_tensor("wvT", [D_MODEL, GD], BF16, kind="ExternalInput")
    woT = nc.dram_tensor("woT", [GD, D_MODEL], BF16, kind="ExternalInput")
    bq = nc.dram_tensor("bq", [P, N_MT], F32, kind="ExternalInput")
    bk = nc.dram_tensor("bk", [P, N_MT], F32, kind="ExternalInput")
    bv = nc.dram_tensor("bv", [1, GD], F32, kind="ExternalInput")
    maskc = nc.dram_tensor("maskc", [P, P], BF16, kind="ExternalInput")
    out = nc.dram_tensor("out", [L, D_MODEL], BF16, kind="ExternalOutput")

    with tile.TileContext(nc) as tc:
        with (
            tc.tile_pool(name="const", bufs=1) as const,
            tc.tile_pool(name="xch", bufs=32) as xch_pool,
            tc.tile_pool(name="xvc", bufs=16) as xvc_pool,
            tc.tile_pool(name="pt", bufs=8) as pt_pool,
            tc.tile_pool(name="aor", bufs=2) as aor_pool,
            tc.tile_pool(name="den", bufs=2) as den_pool,
            tc.tile_pool(name="bc", bufs=2) as bc_pool,
            tc.tile_pool(name="osb", bufs=4) as osb_pool,
            tc.tile_pool(name="dr", bufs=4, space="DRAM") as dr_pool,
            tc.tile_pool(name="ps_s", bufs=2, space="PSUM") as ps_s,
            tc.tile_pool(name="ps_av", bufs=1, space="PSUM") as ps_av,
            tc.tile_pool(name="ps_mm", bufs=2, space="PSUM") as ps_mm,
        ):
            scale = float(1.0 / np.sqrt(np.float32(D_K)))

            # ---- resident weights / constants ----
            wk_sb = const.tile([P, N_KT, GD], BF16, tag="wk")
            wq_sb = const.tile([P, N_KT, GD], BF16, tag="wq")
            wv_sb = const.tile([P, N_KT, GD], BF16, tag="wv")
            wo_sb = const.tile([P, N_MT, D_MODEL], BF16, tag="wo")
            bk_sb = const.tile([P, N_MT], F32, tag="bk")
            bq_sb = const.tile([P, N_MT], F32, tag="bq")
            bv_sb = const.tile([P, GD], F32, tag="bv")
            mask_sb = const.tile([P, P], BF16, tag="mask")

            kTt = [[const.tile([P, NQ], BF16, tag=f"kT{mt}_{c}", name=f"kT{mt}_{c}")
                    for c in range(N_QC)] for mt in range(N_MT)]
            qTt = [[const.tile([P, NQ], BF16, tag=f"qT{mt}_{c}", name=f"qT{mt}_{c}")
                    for c in range(N_QC)] for mt in range(N_MT)]
            vA = [const.tile([P, 2 * N_MT, D_K + 1], BF16, tag=f"v{tt}", name=f"v{tt}")
                  for tt in range(N_TT)]
            aoTq = [[const.tile([P, NQ], BF16, tag=f"ao{mt}_{qc}", name=f"ao{mt}_{qc}")
                     for qc in range(N_QC)] for mt in range(N_MT)]

            def w_dmas():
                # weights on the scalar queue (idle during the prologue) and
                # gpsimd; per-k-tile transfers so the first chains start early
                for kt in range(N_KT):
                    nc.scalar.dma_start(out=wk_sb[:, kt, :],
                                        in_=wkT[kt * P:(kt + 1) * P, :])
                nc.scalar.dma_start(out=bk_sb, in_=bk[:, :])
                nc.scalar.dma_start(out=bq_sb, in_=bq[:, :])
                nc.scalar.dma_start(out=bv_sb, in_=bv[:, :].to_broadcast([P, GD]))
                nc.scalar.dma_start(out=mask_sb, in_=maskc[:, :])
                for kt in range(N_KT):
                    nc.gpsimd.dma_start(out=wv_sb[:, kt, :],
                                        in_=wvT[kt * P:(kt + 1) * P, :])
                for kt in range(N_KT):
                    nc.scalar.dma_start(out=wq_sb[:, kt, :],
                                        in_=wqT[kt * P:(kt + 1) * P, :])
                for mt in range(N_MT):
                    nc.gpsimd.dma_start(out=wo_sb[:, mt, :],
                                        in_=woT[mt * P:(mt + 1) * P, :])

            xch_cache = {}

            def kq_dmas(nm, x_dram, c):
                xchs = []
                for kt in range(N_KT):
                    xc = xch_pool.tile([P, NQ], BF16, tag="xch",
                                       name=f"x{nm}{c}_{kt}")
                    nc.sync.dma_start(
                        out=xc,
                        in_=x_dram[kt * P:(kt + 1) * P, c * NQ:(c + 1) * NQ],
                    )
                    xchs.append(xc)
                xch_cache[(nm, c)] = xchs

            xvc_cache = {}

            def xv_dmas(c):
                xvs = []
                for kt in range(N_KT):
                    xc = xvc_pool.tile([P, NQ], BF16, tag="xvc",
                                       name=f"xv{c}_{kt}")
                    nc.gpsimd.dma_start(
                        out=xc,
                        in_=xvT[kt * P:(kt + 1) * P, c * NQ:(c + 1) * NQ],
                    )
                    xvs.append(xc)
                xvc_cache[c] = xvs

            def kq_part(w_sb, b_sb, dsts, sc, nm, c, mt):
                xchs = xch_cache[(nm, c)]
                ps = ps_mm.tile([P, NQ], F32, tag="mm", name=f"ps{nm}{c}{mt}")
                for kt in range(N_KT):
                    nc.tensor.matmul(
                        ps,
                        lhsT=w_sb[:, kt, mt * P:(mt + 1) * P],
                        rhs=xchs[kt],
                        start=(kt == 0),
                        stop=(kt == N_KT - 1),
                    )
                nc.vector.tensor_scalar(
                    out=dsts[mt][c],
                    in0=ps,
                    scalar1=b_sb[:, mt:mt + 1],
                    scalar2=sc,
                    op0=ALU.add,
                    op1=ALU.mult,
                )

            def v_proj_tile(tt):
                c, j = tt // 4, tt % 4
                xvs = xvc_cache[c]
                ps = ps_mm.tile([P, GD], F32, tag="mm", name=f"psv{tt}")
                for kt in range(N_KT):
                    nc.tensor.matmul(
                        ps,
                        lhsT=xvs[kt][:, j * P:(j + 1) * P],
                        rhs=wv_sb[:, kt, :],
                        start=(kt == 0),
                        stop=(kt == N_KT - 1),
                    )
                nc.vector.tensor_tensor(
                    out=vA[tt][:, :, 0:D_K],
                    in0=ps.rearrange("p (h d) -> p h d", d=D_K),
                    in1=bv_sb.rearrange("p (h d) -> p h d", d=D_K),
                    op=ALU.add,
                )
                nc.vector.memset(vA[tt][:, :, D_K:D_K + 1], 1.0)

            def attention(mt, qc):
                nkb = 4 * qc + 4
                av = ps_av.tile([D_K + 1, 2, NQ], F32, tag="av",
                                name=f"av{mt}_{qc}")
                for kb in range(nkb):
                    t = P * (kb - 4 * qc)  # <0 for full blocks
                    s_ps = ps_s.tile([P, 2 * NQ], F32, tag="s",
                                     name=f"s{mt}_{qc}_{kb}")
                    s3 = s_ps.rearrange("p (h n) -> p h n", n=NQ)
                    for h2 in range(2):
                        nc.tensor.matmul(
                            s3[:, h2, max(t, 0):NQ],
                            lhsT=kTt[mt][kb // 4][h2 * D_K:(h2 + 1) * D_K,
                                                 (kb % 4) * P:(kb % 4 + 1) * P],
                            rhs=qTt[mt][qc][h2 * D_K:(h2 + 1) * D_K,
                                            max(t, 0):NQ],
                            start=True,
                            stop=True,
                        )
                    pt = pt_pool.tile([P, 2 * NQ], BF16, tag="pt",
                                      name=f"pt{mt}_{qc}_{kb}")
                    p3 = pt.rearrange("p (h n) -> p h n", n=NQ)
                    if t <= 0:
                        nc.scalar.activation(out=pt, in_=s_ps, func=AF.Exp)
                    else:
                        nc.scalar.activation(out=p3[:, :, t:NQ],
                                             in_=s3[:, :, t:NQ], func=AF.Exp)
                    if t >= 0:  # diagonal sub-block: triangular mask
                        for h2 in range(2):
                            nc.vector.tensor_tensor(
                                out=p3[:, h2, t:t + P],
                                in0=p3[:, h2, t:t + P],
                                in1=mask_sb,
                                op=ALU.mult,
                            )
                    for h2 in range(2):
                        nc.tensor.matmul(
                            av[:, h2, max(t, 0):NQ],
                            lhsT=vA[kb][:, 2 * mt + h2, :],
                            rhs=p3[:, h2, max(t, 0):NQ],
                            start=(kb == 0),
                            stop=(kb == nkb - 1),
                        )
                # evict av (incl. the ones-row denominators at row 64) in one
                # fast DVE copy so the PSUM bank pair frees for the next mt;
                # the normalization chain below runs off the critical path.
                aor = aor_pool.tile([D_K + 1, 2, NQ], F32, tag="aor",
                                    name=f"aor{mt}_{qc}")
                nc.vector.tensor_copy(out=aor, in_=av)
                # DVE reciprocal costs ~6 cycles per FREE element per lane,
                # so it must run on a [128, few] tile: bounce the den row
                # through DRAM to transpose it across partitions and back.
                den_d = dr_pool.tile([1, 2 * NQ], F32, tag="dend",
                                     name=f"dend{mt}_{qc}")
                nc.gpsimd.dma_start(out=den_d, in_=aor[D_K:D_K + 1, :, :])
                den_t = den_pool.tile([P, 2 * NQ // P], F32, tag="dent",
                                      name=f"dent{mt}_{qc}")
                nc.gpsimd.dma_start(
                    out=den_t,
                    in_=den_d.rearrange("one (p f) -> (one p) f", p=P),
                )
                rec_t = den_pool.tile([P, 2 * NQ // P], F32, tag="rect",
                                      name=f"rect{mt}_{qc}")
                nc.vector.reciprocal(rec_t, den_t)
                rec_d = dr_pool.tile([1, 2 * NQ], F32, tag="recd",
                                     name=f"recd{mt}_{qc}")
                nc.sync.dma_start(
                    out=rec_d.rearrange("one (p f) -> (one p) f", p=P),
                    in_=rec_t,
                )
                bcst = bc_pool.tile([D_K, 2 * NQ], F32, tag="bc",
                                    name=f"bc{mt}_{qc}")
                nc.sync.dma_start(
                    out=bcst,
                    in_=rec_d.to_broadcast([D_K, 2 * NQ]),
                )
                for h2 in range(2):
                    nc.vector.tensor_tensor(
                        out=aoTq[mt][qc][h2 * D_K:(h2 + 1) * D_K, :],
                        in0=aor[0:D_K, h2, :],
                        in1=bcst[:, h2 * NQ:(h2 + 1) * NQ],
                        op=ALU.mult,
                    )

            def o_proj_piece(qc, j, dc):
                lt = 4 * qc + j
                ps = ps_mm.tile([P, NQ], F32, tag="mm", name=f"po{lt}_{dc}")
                for kt in range(N_MT):
                    nc.tensor.matmul(
                        ps,
                        lhsT=aoTq[kt][qc][:, j * P:(j + 1) * P],
                        rhs=wo_sb[:, kt, dc * NQ:(dc + 1) * NQ],
                        start=(kt == 0),
                        stop=(kt == N_MT - 1),
                    )
                ot = osb_pool.tile([P, NQ], BF16, tag="ot", name=f"ot{lt}_{dc}")
                nc.vector.tensor_copy(out=ot, in_=ps)
                nc.sync.dma_start(
                    out=out[lt * P:(lt + 1) * P, dc * NQ:(dc + 1) * NQ],
                    in_=ot,
                )

            # ---------- emission schedule ----------
            w_dmas()
            kq_dmas("k", xkT, 0)
            xv_dmas(0)
            kq_dmas("q", xqT, 0)
            for mt in range(N_MT):
                kq_part(wk_sb, bk_sb, kTt, 1.0, "k", 0, mt)
            for tt in range(4):
                v_proj_tile(tt)
            for mt in range(N_MT):
                kq_part(wq_sb, bq_sb, qTt, scale, "q", 0, mt)

            for qc in range(N_QC):
                if qc < N_QC - 1:  # prefetch next chunk's activations
                    kq_dmas("k", xkT, qc + 1)
                    xv_dmas(qc + 1)
                    kq_dmas("q", xqT, qc + 1)
                if qc > 0:  # k/v of chunk qc feed only this wave's diag blocks
                    for mt in range(N_MT):
                        kq_part(wk_sb, bk_sb, kTt, 1.0, "k", qc, mt)
                    for tt in range(4 * qc, 4 * qc + 4):
                        v_proj_tile(tt)
                for mt in range(N_MT):
                    attention(mt, qc)
                    if qc > 0:  # o_proj of the previous chunk as PE fill
                        o_proj_piece(qc - 1, mt, 0)
                        o_proj_piece(qc - 1, mt, 1)
                    if qc < N_QC - 1:
                        kq_part(wq_sb, bq_sb, qTt, scale, "q", qc + 1, mt)
                if qc == N_QC - 1:
                    for j in range(4):
                        for dc in range(2):
                            o_proj_piece(qc, j, dc)
    nc.finalize()
    return nc


def make_in_maps(Q, K, V, Wq, bq, Wk, bk, Wv, bv, Wo, bo, attn_mask=None):
    """Build the 8 per-core input maps from full (unsharded) inputs."""
    Q = np.asarray(Q, np.float32)
    K = np.asarray(K, np.float32)
    V = np.asarray(V, np.float32)
    Wq = np.asarray(Wq, np.float32)
    Wk = np.asarray(Wk, np.float32)
    Wv = np.asarray(Wv, np.float32)
    Wo = np.asarray(Wo, np.float32)
    bq = np.asarray(bq, np.float32)
    bk = np.asarray(bk, np.float32)
    bv = np.asarray(bv, np.float32)

    i_idx = np.arange(P)[:, None]
    j_idx = np.arange(P)[None, :]
    maskc = (i_idx <= j_idx).astype(NPBF16)

    xT = {}
    for b in range(B):
        xT[b] = tuple(
            np.ascontiguousarray(X[b].T).astype(NPBF16) for X in (Q, K, V)
        )
    grp = {}
    for g in range(TP):
        sl = slice(g * GD, (g + 1) * GD)
        grp[g] = dict(
            wqT=np.ascontiguousarray(Wq[sl, :].T).astype(NPBF16),
            wkT=np.ascontiguousarray(Wk[sl, :].T).astype(NPBF16),
            wvT=np.ascontiguousarray(Wv[sl, :].T).astype(NPBF16),
            woT=np.ascontiguousarray(Wo[:, sl].T).astype(NPBF16),
            bq=np.ascontiguousarray(bq[sl].reshape(N_MT, P).T).astype(np.float32),
            bk=np.ascontiguousarray(bk[sl].reshape(N_MT, P).T).astype(np.float32),
            bv=np.ascontiguousarray(bv[sl].reshape(1, GD)).astype(np.float32),
        )
    in_maps = []
    for c in range(2 * B):
        b, g = c // 2, c % 2
        m = dict(grp[g])
        m["xqT"], m["xkT"], m["xvT"] = xT[b]
        m["maskc"] = maskc
        in_maps.append(m)
    return in_maps


def assemble_output(results, bo):
    bo = np.asarray(bo, np.float32)
    out = np.empty((B, L, D_MODEL), np.float32)
    for b in range(B):
        out[b] = (results[2 * b]["out"].astype(np.float32)
                  + results[2 * b + 1]["out"].astype(np.float32) + bo)
    return out


_NC_CACHE = None


def kernel(**inputs) -> np.ndarray:
    global _NC_CACHE
    from concourse.bass_utils import run_bass_kernel_spmd

    if _NC_CACHE is None:
        _NC_CACHE = build_nc()
    in_maps = make_in_maps(**inputs)
    res = run_bass_kernel_spmd(_NC_CACHE, in_maps, core_ids=list(range(2 * B)))
    return assemble_output(res.results, inputs["bo"])


# revision 8
# speedup vs baseline: 1.0016x; 1.0016x over previous
"""Multi-head causal attention (QKV proj + attention + O proj) on 8 TRN2 cores.

Sharding: data-parallel over batch (4) x tensor-parallel over heads (2 groups
of 8 heads).  Core c handles batch c//2, head-group c%2.  Each core computes
its group's partial o_proj output; the host sums the two partials per batch.

Layout strategy (all activations arrive pre-transposed from the host, so the
kernel never transposes on-device):
  - qT, kT per head-pair M-tile: (128 head-dims, L) from  W.T-slice @ X.T
  - v natural (tokens, head-dims) with a fused ones-column for the softmax
    denominator: av_psum = v_aug.T @ P.T gives (65, 2, Nq) where row 64 is
    the per-query sum of probabilities.
  - scores are computed transposed (keys on partitions, queries free); the
    two heads of an M-tile use PE row-tiles (0,0)/(64,0) and run concurrent.
  - av accumulates over ALL key blocks of a query chunk in one PSUM bank
    pair (no SBUF accumulator).  Softmax denominators: DVE reciprocal of the
    ones-row, GpSimd partition_broadcast across the 64 head dims, DVE mult.
  - causal masking multiplies a small triangular mask after exp on diagonal
    128-blocks; fully-masked ranges are never computed.
Compute dtype bf16 (fp32 PSUM accumulation); bf16 partial outputs summed in
fp32 on the host.

Schedule: wave per query chunk qc.  Wave qc runs attention(qc) interleaved
with k/v projections of chunk qc (needed only by the diagonal blocks),
q projection of chunk qc+1, o_proj of chunk qc-1, and x-chunk prefetch, so
the PE-bound projections fill the scalar-bound (exp) attention phases and
output DMA is spread across the whole kernel instead of a tail wave.
"""

import numpy as np
import ml_dtypes

import concourse.bass as bass
import concourse.tile as tile
from concourse import bacc, mybir

D_MODEL = 1024
N_HEADS = 16
D_K = 64
B, L = 4, 2048
TP = 2                  # head groups
GD = D_MODEL // TP      # 512 head-dims per group
P = 128
NQ = 512                # query chunk (one fp32 PSUM bank)
N_MT = GD // P          # 4 M-tiles (head pairs) per group
N_KT = D_MODEL // P     # 8 contraction tiles over model dim
N_TT = L // P           # 16 token tiles
N_QC = L // NQ          # 4 query chunks
BF16 = mybir.dt.bfloat16
F32 = mybir.dt.float32
NPBF16 = ml_dtypes.bfloat16
AF = mybir.ActivationFunctionType
ALU = mybir.AluOpType


def build_nc() -> bass.Bass:
    nc = bacc.Bacc("TRN2", target_bir_lowering=False)

    xqT = nc.dram_tensor("xqT", [D_MODEL, L], BF16, kind="ExternalInput")
    xkT = nc.dram_tensor("xkT", [D_MODEL, L], BF16, kind="ExternalInput")
    xvT = nc.dram_tensor("xvT", [D_MODEL, L], BF16, kind="ExternalInput")
    wqT = nc.dram_tensor("wqT", [D_MODEL, GD], BF16, kind="ExternalInput")
    wkT = nc.dram_tensor("wkT", [D_MODEL, GD], BF16, kind="ExternalInput")
    wvT = nc.dram_tensor("wvT", [D_MODEL, GD], BF16, kind="ExternalInput")
    woT = nc.dram_tensor("woT", [GD, D_MODEL], BF16, kind="ExternalInput")
    bq = nc.dram_tensor("bq", [P, N_MT], F32, kind="ExternalInput")
    bk = nc.dram_tensor("bk", [P, N_MT], F32, kind="ExternalInput")
    bv = nc.dram_tensor("bv", [1, GD], F32, kind="ExternalInput")
    maskc = nc.dram_tensor("maskc", [P, P], BF16, kind="ExternalInput")
    out = nc.dram_tensor("out", [L, D_MODEL], F32, kind="ExternalOutput")

    with tile.TileContext(nc) as tc:
        with (
            tc.tile_pool(name="const", bufs=1) as const,
            tc.tile_pool(name="xch", bufs=32) as xch_pool,
            tc.tile_pool(name="xvc", bufs=16) as xvc_pool,
            tc.tile_pool(name="pt", bufs=8) as pt_pool,
            tc.tile_pool(name="aor", bufs=2) as aor_pool,
            tc.tile_pool(name="den", bufs=2) as den_pool,
            tc.tile_pool(name="bc", bufs=2) as bc_pool,
            tc.tile_pool(name="osb", bufs=4) as osb_pool,
            tc.tile_pool(name="dr", bufs=4, space="DRAM") as dr_pool,
            tc.tile_pool(name="ps_s", bufs=2, space="PSUM") as ps_s,
            tc.tile_pool(name="ps_av", bufs=1, space="PSUM") as ps_av,
            tc.tile_pool(name="ps_mm", bufs=2, space="PSUM") as ps_mm,
        ):
            scale = float(1.0 / np.sqrt(np.float32(D_K)))

            # ---- resident weights / constants ----
            wk_sb = const.tile([P, N_KT, GD], BF16, tag="wk")
            wq_sb = const.tile([P, N_KT, GD], BF16, tag="wq")
            wv_sb = const.tile([P, N_KT, GD], BF16, tag="wv")
            wo_sb = const.tile([P, N_MT, D_MODEL], BF16, tag="wo")
            bk_sb = const.tile([P, N_MT], F32, tag="bk")
            bq_sb = const.tile([P, N_MT], F32, tag="bq")
            bv_sb = const.tile([P, GD], F32, tag="bv")
            mask_sb = const.tile([P, P], BF16, tag="mask")

            kTt = [[const.tile([P, NQ], BF16, tag=f"kT{mt}_{c}", name=f"kT{mt}_{c}")
                    for c in range(N_QC)] for mt in range(N_MT)]
            qTt = [[const.tile([P, NQ], BF16, tag=f"qT{mt}_{c}", name=f"qT{mt}_{c}")
                    for c in range(N_QC)] for mt in range(N_MT)]
            vA = [const.tile([P, 2 * N_MT, D_K + 1], BF16, tag=f"v{tt}", name=f"v{tt}")
                  for tt in range(N_TT)]
            aoTq = [[const.tile([P, NQ], BF16, tag=f"ao{mt}_{qc}", name=f"ao{mt}_{qc}")
                     for qc in range(N_QC)] for mt in range(N_MT)]

            def w_dmas():
                # weights on the scalar queue (idle during the prologue) and
                # gpsimd; per-k-tile transfers so the first chains start early
                for kt in range(N_KT):
                    nc.scalar.dma_start(out=wk_sb[:, kt, :],
                                        in_=wkT[kt * P:(kt + 1) * P, :])
                nc.scalar.dma_start(out=bk_sb, in_=bk[:, :])
                nc.scalar.dma_start(out=bq_sb, in_=bq[:, :])
                nc.scalar.dma_start(out=bv_sb, in_=bv[:, :].to_broadcast([P, GD]))
                nc.scalar.dma_start(out=mask_sb, in_=maskc[:, :])
                for kt in range(N_KT):
                    nc.gpsimd.dma_start(out=wv_sb[:, kt, :],
                                        in_=wvT[kt * P:(kt + 1) * P, :])
                for kt in range(N_KT):
                    nc.scalar.dma_start(out=wq_sb[:, kt, :],
                                        in_=wqT[kt * P:(kt + 1) * P, :])
                for mt in range(N_MT):
                    nc.gpsimd.dma_start(out=wo_sb[:, mt, :],
                                        in_=woT[mt * P:(mt + 1) * P, :])

            xch_cache = {}

            def kq_dmas(nm, x_dram, c):
                xchs = []
                for kt in range(N_KT):
                    xc = xch_pool.tile([P, NQ], BF16, tag="xch",
                                       name=f"x{nm}{c}_{kt}")
                    nc.sync.dma_start(
                        out=xc,
                        in_=x_dram[kt * P:(kt + 1) * P, c * NQ:(c + 1) * NQ],
                    )
                    xchs.append(xc)
                xch_cache[(nm, c)] = xchs

            xvc_cache = {}

            def xv_dmas(c):
                xvs = []
                for kt in range(N_KT):
                    xc = xvc_pool.tile([P, NQ], BF16, tag="xvc",
                                       name=f"xv{c}_{kt}")
                    nc.gpsimd.dma_start(
                        out=xc,
                        in_=xvT[kt * P:(kt + 1) * P, c * NQ:(c + 1) * NQ],
                    )
                    xvs.append(xc)
                xvc_cache[c] = xvs

            def kq_part(w_sb, b_sb, dsts, sc, nm, c, mt):
                xchs = xch_cache[(nm, c)]
                ps = ps_mm.tile([P, NQ], F32, tag="mm", name=f"ps{nm}{c}{mt}")
                for kt in range(N_KT):
                    nc.tensor.matmul(
                        ps,
                        lhsT=w_sb[:, kt, mt * P:(mt + 1) * P],
                        rhs=xchs[kt],
                        start=(kt == 0),
                        stop=(kt == N_KT - 1),
                    )
                nc.vector.tensor_scalar(
                    out=dsts[mt][c],
                    in0=ps,
                    scalar1=b_sb[:, mt:mt + 1],
                    scalar2=sc,
                    op0=ALU.add,
                    op1=ALU.mult,
                )

            def v_proj_tile(tt):
                c, j = tt // 4, tt % 4
                xvs = xvc_cache[c]
                ps = ps_mm.tile([P, GD], F32, tag="mm", name=f"psv{tt}")
                for kt in range(N_KT):
                    nc.tensor.matmul(
                        ps,
                        lhsT=xvs[kt][:, j * P:(j + 1) * P],
                        rhs=wv_sb[:, kt, :],
                        start=(kt == 0),
                        stop=(kt == N_KT - 1),
                    )
                nc.vector.tensor_tensor(
                    out=vA[tt][:, :, 0:D_K],
                    in0=ps.rearrange("p (h d) -> p h d", d=D_K),
                    in1=bv_sb.rearrange("p (h d) -> p h d", d=D_K),
                    op=ALU.add,
                )
                nc.vector.memset(vA[tt][:, :, D_K:D_K + 1], 1.0)

            def attention(mt, qc):
                nkb = 4 * qc + 4
                av = ps_av.tile([D_K + 1, 2, NQ], F32, tag="av",
                                name=f"av{mt}_{qc}")
                for kb in range(nkb):
                    t = P * (kb - 4 * qc)  # <0 for full blocks
                    s_ps = ps_s.tile([P, 2 * NQ], F32, tag="s",
                                     name=f"s{mt}_{qc}_{kb}")
                    s3 = s_ps.rearrange("p (h n) -> p h n", n=NQ)
                    for h2 in range(2):
                        nc.tensor.matmul(
                            s3[:, h2, max(t, 0):NQ],
                            lhsT=kTt[mt][kb // 4][h2 * D_K:(h2 + 1) * D_K,
                                                 (kb % 4) * P:(kb % 4 + 1) * P],
                            rhs=qTt[mt][qc][h2 * D_K:(h2 + 1) * D_K,
                                            max(t, 0):NQ],
                            start=True,
                            stop=True,
                        )
                    pt = pt_pool.tile([P, 2 * NQ], BF16, tag="pt",
                                      name=f"pt{mt}_{qc}_{kb}")
                    p3 = pt.rearrange("p (h n) -> p h n", n=NQ)
                    if t <= 0:
                        nc.scalar.activation(out=pt, in_=s_ps, func=AF.Exp)
                    else:
                        nc.scalar.activation(out=p3[:, :, t:NQ],
                                             in_=s3[:, :, t:NQ], func=AF.Exp)
                    if t >= 0:  # diagonal sub-block: triangular mask
                        for h2 in range(2):
                            nc.vector.tensor_tensor(
                                out=p3[:, h2, t:t + P],
                                in0=p3[:, h2, t:t + P],
                                in1=mask_sb,
                                op=ALU.mult,
                            )
                    for h2 in range(2):
                        nc.tensor.matmul(
                            av[:, h2, max(t, 0):NQ],
                            lhsT=vA[kb][:, 2 * mt + h2, :],
                            rhs=p3[:, h2, max(t, 0):NQ],
                            start=(kb == 0),
                            stop=(kb == nkb - 1),
                        )
                # evict av (incl. the ones-row denominators at row 64) in one
                # fast DVE copy so the PSUM bank pair frees for the next mt;
                # the normalization chain below runs off the critical path.
                aor = aor_pool.tile([D_K + 1, 2, NQ], F32, tag="aor",
                                    name=f"aor{mt}_{qc}")
                nc.vector.tensor_copy(out=aor, in_=av)
                # DVE reciprocal costs ~6 cycles per FREE element per lane,
                # so it must run on a [128, few] tile: bounce the den row
                # through DRAM to transpose it across partitions and back.
                den_d = dr_pool.tile([1, 2 * NQ], F32, tag="dend",
                                     name=f"dend{mt}_{qc}")
                nc.gpsimd.dma_start(out=den_d, in_=aor[D_K:D_K + 1, :, :])
                den_t = den_pool.tile([P, 2 * NQ // P], F32, tag="dent",
                                      name=f"dent{mt}_{qc}")
                nc.gpsimd.dma_start(
                    out=den_t,
                    in_=den_d.rearrange("one (p f) -> (one p) f", p=P),
                )
                rec_t = den_pool.tile([P, 2 * NQ // P], F32, tag="rect",
                                      name=f"rect{mt}_{qc}")
                nc.vector.reciprocal(rec_t, den_t)
                rec_d = dr_pool.tile([1, 2 * NQ], F32, tag="recd",
                                     name=f"recd{mt}_{qc}")
                nc.sync.dma_start(
                    out=rec_d.rearrange("one (p f) -> (one p) f", p=P),
                    in_=rec_t,
                )
                bcst = bc_pool.tile([D_K, 2 * NQ], F32, tag="bc",
                                    name=f"bc{mt}_{qc}")
                nc.sync.dma_start(
                    out=bcst,
                    in_=rec_d.to_broadcast([D_K, 2 * NQ]),
                )
                for h2 in range(2):
                    nc.vector.tensor_tensor(
                        out=aoTq[mt][qc][h2 * D_K:(h2 + 1) * D_K, :],
                        in0=aor[0:D_K, h2, :],
                        in1=bcst[:, h2 * NQ:(h2 + 1) * NQ],
                        op=ALU.mult,
                    )

            def o_proj_piece(qc, j, dc):
                lt = 4 * qc + j
                ps = ps_mm.tile([P, NQ], F32, tag="mm", name=f"po{lt}_{dc}")
                for kt in range(N_MT):
                    nc.tensor.matmul(
                        ps,
                        lhsT=aoTq[kt][qc][:, j * P:(j + 1) * P],
                        rhs=wo_sb[:, kt, dc * NQ:(dc + 1) * NQ],
                        start=(kt == 0),
                        stop=(kt == N_MT - 1),
                    )
                ot = osb_pool.tile([P, NQ], F32, tag="ot", name=f"ot{lt}_{dc}")
                nc.vector.tensor_copy(out=ot, in_=ps)
                nc.sync.dma_start(
                    out=out[lt * P:(lt + 1) * P, dc * NQ:(dc + 1) * NQ],
                    in_=ot,
                )

            # ---------- emission schedule ----------
            w_dmas()
            kq_dmas("k", xkT, 0)
            xv_dmas(0)
            kq_dmas("q", xqT, 0)
            for mt in range(N_MT):
                kq_part(wk_sb, bk_sb, kTt, 1.0, "k", 0, mt)
            for tt in range(4):
                v_proj_tile(tt)
            for mt in range(N_MT):
                kq_part(wq_sb, bq_sb, qTt, scale, "q", 0, mt)

            for qc in range(N_QC):
                if qc < N_QC - 1:  # prefetch next chunk's activations
                    kq_dmas("k", xkT, qc + 1)
                    xv_dmas(qc + 1)
                    kq_dmas("q", xqT, qc + 1)
                if qc > 0:  # k/v of chunk qc feed only this wave's diag blocks
                    for mt in range(N_MT):
                        kq_part(wk_sb, bk_sb, kTt, 1.0, "k", qc, mt)
                    for tt in range(4 * qc, 4 * qc + 4):
                        v_proj_tile(tt)
                for mt in range(N_MT):
                    attention(mt, qc)
                    if qc > 0:  # o_proj of the previous chunk as PE fill
                        o_proj_piece(qc - 1, mt, 0)
                        o_proj_piece(qc - 1, mt, 1)
                    if qc < N_QC - 1:
                        kq_part(wq_sb, bq_sb, qTt, scale, "q", qc + 1, mt)
                if qc == N_QC - 1:
                    for j in range(4):
                        for dc in range(2):
                            o_proj_piece(qc, j, dc)
    nc.finalize()
    return nc


def make_in_maps(Q, K, V, Wq, bq, Wk, bk, Wv, bv, Wo, bo, attn_mask=None):
    """Build the 8 per-core input maps from full (unsharded) inputs."""
    Q = np.asarray(Q, np.float32)
    K = np.asarray(K, np.float32)
    V = np.asarray(V, np.float32)
    Wq = np.asarray(Wq, np.float32)
    Wk = np.asarray(Wk, np.float32)
    Wv = np.asarray(Wv, np.float32)
    Wo = np.asarray(Wo, np.float32)
    bq = np.asarray(bq, np.float32)
    bk = np.asarray(bk, np.float32)
    bv = np.asarray(bv, np.float32)

    i_idx = np.arange(P)[:, None]
    j_idx = np.arange(P)[None, :]
    maskc = (i_idx <= j_idx).astype(NPBF16)

    xT = {}
    for b in range(B):
        xT[b] = tuple(
            np.ascontiguousarray(X[b].T).astype(NPBF16) for X in (Q, K, V)
        )
    grp = {}
    for g in range(TP):
        sl = slice(g * GD, (g + 1) * GD)
        grp[g] = dict(
            wqT=np.ascontiguousarray(Wq[sl, :].T).astype(NPBF16),
            wkT=np.ascontiguousarray(Wk[sl, :].T).astype(NPBF16),
            wvT=np.ascontiguousarray(Wv[sl, :].T).astype(NPBF16),
            woT=np.ascontiguousarray(Wo[:, sl].T).astype(NPBF16),
            bq=np.ascontiguousarray(bq[sl].reshape(N_MT, P).T).astype(np.float32),
            bk=np.ascontiguousarray(bk[sl].reshape(N_MT, P).T).astype(np.float32),
            bv=np.ascontiguousarray(bv[sl].reshape(1, GD)).astype(np.float32),
        )
    in_maps = []
    for c in range(2 * B):
        b, g = c // 2, c % 2
        m = dict(grp[g])
        m["xqT"], m["xkT"], m["xvT"] = xT[b]
        m["maskc"] = maskc
        in_maps.append(m)
    return in_maps


def assemble_output(results, bo):
    bo = np.asarray(bo, np.float32)
    out = np.empty((B, L, D_MODEL), np.float32)
    for b in range(B):
        out[b] = (results[2 * b]["out"].astype(np.float32)
                  + results[2 * b + 1]["out"].astype(np.float32) + bo)
    return out


_NC_CACHE = None


def kernel(**inputs) -> np.ndarray:
    global _NC_CACHE
    from concourse.bass_utils import run_bass_kernel_spmd

    if _NC_CACHE is None:
        _NC_CACHE = build_nc()
    in_maps = make_in_maps(**inputs)
    res = run_bass_kernel_spmd(_NC_CACHE, in_maps, core_ids=list(range(2 * B)))
    return assemble_output(res.results, inputs["bo"])


# revision 11
# speedup vs baseline: 1.0096x; 1.0079x over previous
"""Multi-head causal attention (QKV proj + attention + O proj) on 8 TRN2 cores.

Sharding: data-parallel over batch (4) x tensor-parallel over heads (2 groups
of 8 heads).  Core c handles batch c//2, head-group c%2.  Each core computes
its group's partial o_proj output; the host sums the two partials per batch.

Layout strategy (all activations arrive pre-transposed from the host, so the
kernel never transposes on-device):
  - qT, kT per head-pair M-tile: (128 head-dims, L) from  W.T-slice @ X.T
  - v natural (tokens, head-dims) with a fused ones-column for the softmax
    denominator: av_psum = v_aug.T @ P.T gives (65, 2, Nq) where row 64 is
    the per-query sum of probabilities.
  - scores are computed transposed (keys on partitions, queries free); the
    two heads of an M-tile use PE row-tiles (0,0)/(64,0) and run concurrent.
  - av accumulates over ALL key blocks of a query chunk in one PSUM bank
    pair (no SBUF accumulator).  Softmax denominators: DVE reciprocal of the
    ones-row, GpSimd partition_broadcast across the 64 head dims, DVE mult.
  - causal masking multiplies a small triangular mask after exp on diagonal
    128-blocks; fully-masked ranges are never computed.
Compute dtype bf16 (fp32 PSUM accumulation); bf16 partial outputs summed in
fp32 on the host.

Schedule: wave per query chunk qc.  Wave qc runs attention(qc) interleaved
with k/v projections of chunk qc (needed only by the diagonal blocks),
q projection of chunk qc+1, o_proj of chunk qc-1, and x-chunk prefetch, so
the PE-bound projections fill the scalar-bound (exp) attention phases and
output DMA is spread across the whole kernel instead of a tail wave.
"""

import numpy as np
import ml_dtypes

import concourse.bass as bass
import concourse.tile as tile
from concourse import bacc, mybir

D_MODEL = 1024
N_HEADS = 16
D_K = 64
B, L = 4, 2048
TP = 2                  # head groups
GD = D_MODEL // TP      # 512 head-dims per group
P = 128
NQ = 512                # query chunk (one fp32 PSUM bank)
N_MT = GD // P          # 4 M-tiles (head pairs) per group
N_KT = D_MODEL // P     # 8 contraction tiles over model dim
N_TT = L // P           # 16 token tiles
N_QC = L // NQ          # 4 query chunks
BF16 = mybir.dt.bfloat16
F32 = mybir.dt.float32
NPBF16 = ml_dtypes.bfloat16
AF = mybir.ActivationFunctionType
ALU = mybir.AluOpType


def build_nc() -> bass.Bass:
    nc = bacc.Bacc("TRN2", target_bir_lowering=False)

    xqT = nc.dram_tensor("xqT", [D_MODEL, L], BF16, kind="ExternalInput")
    xkT = nc.dram_tensor("xkT", [D_MODEL, L], BF16, kind="ExternalInput")
    xvT = nc.dram_tensor("xvT", [D_MODEL, L], BF16, kind="ExternalInput")
    wqT = nc.dram_tensor("wqT", [D_MODEL, GD], BF16, kind="ExternalInput")
    wkT = nc.dram_tensor("wkT", [D_MODEL, GD], BF16, kind="ExternalInput")
    wvT = nc.dram_tensor("wvT", [D_MODEL, GD], BF16, kind="ExternalInput")
    woT = nc.dram_tensor("woT", [GD, D_MODEL], BF16, kind="ExternalInput")
    bq = nc.dram_tensor("bq", [P, N_MT], F32, kind="ExternalInput")
    bk = nc.dram_tensor("bk", [P, N_MT], F32, kind="ExternalInput")
    bv = nc.dram_tensor("bv", [1, GD], F32, kind="ExternalInput")
    maskc = nc.dram_tensor("maskc", [P, P], BF16, kind="ExternalInput")
    out = nc.dram_tensor("out", [L, D_MODEL], F32, kind="ExternalOutput")

    with tile.TileContext(nc) as tc:
        with (
            tc.tile_pool(name="const", bufs=1) as const,
            tc.tile_pool(name="xch", bufs=32) as xch_pool,
            tc.tile_pool(name="xvc", bufs=16) as xvc_pool,
            tc.tile_pool(name="pt", bufs=8) as pt_pool,
            tc.tile_pool(name="aor", bufs=2) as aor_pool,
            tc.tile_pool(name="den", bufs=2) as den_pool,
            tc.tile_pool(name="bc", bufs=2) as bc_pool,
            tc.tile_pool(name="osb", bufs=4) as osb_pool,
            tc.tile_pool(name="dr", bufs=4, space="DRAM") as dr_pool,
            tc.tile_pool(name="ps_s", bufs=2, space="PSUM") as ps_s,
            tc.tile_pool(name="ps_av", bufs=1, space="PSUM") as ps_av,
            tc.tile_pool(name="ps_mm", bufs=2, space="PSUM") as ps_mm,
        ):
            scale = float(1.0 / np.sqrt(np.float32(D_K)))

            # ---- resident weights / constants ----
            wk_sb = const.tile([P, N_KT, GD], BF16, tag="wk")
            wq_sb = const.tile([P, N_KT, GD], BF16, tag="wq")
            wv_sb = const.tile([P, N_KT, GD], BF16, tag="wv")
            wo_sb = const.tile([P, N_MT, D_MODEL], BF16, tag="wo")
            bk_sb = const.tile([P, N_MT], F32, tag="bk")
            bq_sb = const.tile([P, N_MT], F32, tag="bq")
            bv_sb = const.tile([P, GD], F32, tag="bv")
            mask_sb = const.tile([P, P], BF16, tag="mask")

            kTt = [[const.tile([P, NQ], BF16, tag=f"kT{mt}_{c}", name=f"kT{mt}_{c}")
                    for c in range(N_QC)] for mt in range(N_MT)]
            qTt = [[const.tile([P, NQ], BF16, tag=f"qT{mt}_{c}", name=f"qT{mt}_{c}")
                    for c in range(N_QC)] for mt in range(N_MT)]
            vA = [const.tile([P, 2 * N_MT, D_K + 1], BF16, tag=f"v{tt}", name=f"v{tt}")
                  for tt in range(N_TT)]
            aoTq = [[const.tile([P, NQ], BF16, tag=f"ao{mt}_{qc}", name=f"ao{mt}_{qc}")
                     for qc in range(N_QC)] for mt in range(N_MT)]

            def w_dmas():
                # spread the startup-critical transfers (wk + xk chunk 0,
                # loaded by the caller on sync) over four queues so the first
                # k-projection chains are fed within a few microseconds
                for kt in range(N_KT):
                    eng = nc.scalar if kt < 4 else nc.gpsimd
                    eng.dma_start(out=wk_sb[:, kt, :],
                                  in_=wkT[kt * P:(kt + 1) * P, :])
                nc.scalar.dma_start(out=bk_sb, in_=bk[:, :])
                nc.scalar.dma_start(out=bq_sb, in_=bq[:, :])
                nc.scalar.dma_start(out=bv_sb, in_=bv[:, :].to_broadcast([P, GD]))
                nc.scalar.dma_start(out=mask_sb, in_=maskc[:, :])
                for kt in range(N_KT):
                    nc.gpsimd.dma_start(out=wv_sb[:, kt, :],
                                        in_=wvT[kt * P:(kt + 1) * P, :])
                for kt in range(N_KT):
                    nc.scalar.dma_start(out=wq_sb[:, kt, :],
                                        in_=wqT[kt * P:(kt + 1) * P, :])
                for mt in range(N_MT):
                    nc.gpsimd.dma_start(out=wo_sb[:, mt, :],
                                        in_=woT[mt * P:(mt + 1) * P, :])

            xch_cache = {}

            def kq_dmas(nm, x_dram, c):
                xchs = []
                for kt in range(N_KT):
                    xc = xch_pool.tile([P, NQ], BF16, tag="xch",
                                       name=f"x{nm}{c}_{kt}")
                    nc.sync.dma_start(
                        out=xc,
                        in_=x_dram[kt * P:(kt + 1) * P, c * NQ:(c + 1) * NQ],
                    )
                    xchs.append(xc)
                xch_cache[(nm, c)] = xchs

            xvc_cache = {}

            def xv_dmas(c):
                xvs = []
                for kt in range(N_KT):
                    xc = xvc_pool.tile([P, NQ], BF16, tag="xvc",
                                       name=f"xv{c}_{kt}")
                    nc.gpsimd.dma_start(
                        out=xc,
                        in_=xvT[kt * P:(kt + 1) * P, c * NQ:(c + 1) * NQ],
                    )
                    xvs.append(xc)
                xvc_cache[c] = xvs

            def kq_part(w_sb, b_sb, dsts, sc, nm, c, mt):
                xchs = xch_cache[(nm, c)]
                ps = ps_mm.tile([P, NQ], F32, tag="mm", name=f"ps{nm}{c}{mt}")
                for kt in range(N_KT):
                    nc.tensor.matmul(
                        ps,
                        lhsT=w_sb[:, kt, mt * P:(mt + 1) * P],
                        rhs=xchs[kt],
                        start=(kt == 0),
                        stop=(kt == N_KT - 1),
                    )
                nc.vector.tensor_scalar(
                    out=dsts[mt][c],
                    in0=ps,
                    scalar1=b_sb[:, mt:mt + 1],
                    scalar2=sc,
                    op0=ALU.add,
                    op1=ALU.mult,
                )

            def v_proj_tile(tt):
                c, j = tt // 4, tt % 4
                xvs = xvc_cache[c]
                ps = ps_mm.tile([P, GD], F32, tag="mm", name=f"psv{tt}")
                for kt in range(N_KT):
                    nc.tensor.matmul(
                        ps,
                        lhsT=xvs[kt][:, j * P:(j + 1) * P],
                        rhs=wv_sb[:, kt, :],
                        start=(kt == 0),
                        stop=(kt == N_KT - 1),
                    )
                nc.vector.tensor_tensor(
                    out=vA[tt][:, :, 0:D_K],
                    in0=ps.rearrange("p (h d) -> p h d", d=D_K),
                    in1=bv_sb.rearrange("p (h d) -> p h d", d=D_K),
                    op=ALU.add,
                )
                nc.vector.memset(vA[tt][:, :, D_K:D_K + 1], 1.0)

            def attention(mt, qc):
                nkb = 4 * qc + 4
                av = ps_av.tile([D_K + 1, 2, NQ], F32, tag="av",
                                name=f"av{mt}_{qc}")
                for kb in range(nkb):
                    t = P * (kb - 4 * qc)  # <0 for full blocks
                    s_ps = ps_s.tile([P, 2 * NQ], F32, tag="s",
                                     name=f"s{mt}_{qc}_{kb}")
                    s3 = s_ps.rearrange("p (h n) -> p h n", n=NQ)
                    for h2 in range(2):
                        nc.tensor.matmul(
                            s3[:, h2, max(t, 0):NQ],
                            lhsT=kTt[mt][kb // 4][h2 * D_K:(h2 + 1) * D_K,
                                                 (kb % 4) * P:(kb % 4 + 1) * P],
                            rhs=qTt[mt][qc][h2 * D_K:(h2 + 1) * D_K,
                                            max(t, 0):NQ],
                            start=True,
                            stop=True,
                        )
                    pt = pt_pool.tile([P, 2 * NQ], BF16, tag="pt",
                                      name=f"pt{mt}_{qc}_{kb}")
                    p3 = pt.rearrange("p (h n) -> p h n", n=NQ)
                    if t <= 0:
                        nc.scalar.activation(out=pt, in_=s_ps, func=AF.Exp)
                    else:
                        nc.scalar.activation(out=p3[:, :, t:NQ],
                                             in_=s3[:, :, t:NQ], func=AF.Exp)
                    if t >= 0:  # diagonal sub-block: triangular mask
                        for h2 in range(2):
                            nc.vector.tensor_tensor(
                                out=p3[:, h2, t:t + P],
                                in0=p3[:, h2, t:t + P],
                                in1=mask_sb,
                                op=ALU.mult,
                            )
                    for h2 in range(2):
                        nc.tensor.matmul(
                            av[:, h2, max(t, 0):NQ],
                            lhsT=vA[kb][:, 2 * mt + h2, :],
                            rhs=p3[:, h2, max(t, 0):NQ],
                            start=(kb == 0),
                            stop=(kb == nkb - 1),
                        )
                # evict av (incl. the ones-row denominators at row 64) in one
                # fast DVE copy so the PSUM bank pair frees for the next mt;
                # the normalization chain below runs off the critical path.
                aor = aor_pool.tile([D_K + 1, 2, NQ], F32, tag="aor",
                                    name=f"aor{mt}_{qc}")
                nc.vector.tensor_copy(out=aor, in_=av)
                # DVE reciprocal costs ~6 cycles per FREE element per lane,
                # so it must run on a [128, few] tile: bounce the den row
                # through DRAM to transpose it across partitions and back.
                den_d = dr_pool.tile([1, 2 * NQ], F32, tag="dend",
                                     name=f"dend{mt}_{qc}")
                nc.gpsimd.dma_start(out=den_d, in_=aor[D_K:D_K + 1, :, :])
                den_t = den_pool.tile([P, 2 * NQ // P], F32, tag="dent",
                                      name=f"dent{mt}_{qc}")
                nc.gpsimd.dma_start(
                    out=den_t,
                    in_=den_d.rearrange("one (p f) -> (one p) f", p=P),
                )
                rec_t = den_pool.tile([P, 2 * NQ // P], F32, tag="rect",
                                      name=f"rect{mt}_{qc}")
                nc.vector.reciprocal(rec_t, den_t)
                rec_d = dr_pool.tile([1, 2 * NQ], F32, tag="recd",
                                     name=f"recd{mt}_{qc}")
                nc.sync.dma_start(
                    out=rec_d.rearrange("one (p f) -> (one p) f", p=P),
                    in_=rec_t,
                )
                bcst = bc_pool.tile([D_K, 2 * NQ], F32, tag="bc",
                                    name=f"bc{mt}_{qc}")
                nc.sync.dma_start(
                    out=bcst,
                    in_=rec_d.to_broadcast([D_K, 2 * NQ]),
                )
                for h2 in range(2):
                    nc.vector.tensor_tensor(
                        out=aoTq[mt][qc][h2 * D_K:(h2 + 1) * D_K, :],
                        in0=aor[0:D_K, h2, :],
                        in1=bcst[:, h2 * NQ:(h2 + 1) * NQ],
                        op=ALU.mult,
                    )

            def o_proj_piece(qc, j, dc):
                lt = 4 * qc + j
                ps = ps_mm.tile([P, NQ], F32, tag="mm", name=f"po{lt}_{dc}")
                for kt in range(N_MT):
                    nc.tensor.matmul(
                        ps,
                        lhsT=aoTq[kt][qc][:, j * P:(j + 1) * P],
                        rhs=wo_sb[:, kt, dc * NQ:(dc + 1) * NQ],
                        start=(kt == 0),
                        stop=(kt == N_MT - 1),
                    )
                ot = osb_pool.tile([P, NQ], F32, tag="ot", name=f"ot{lt}_{dc}")
                nc.vector.tensor_copy(out=ot, in_=ps)
                nc.sync.dma_start(
                    out=out[lt * P:(lt + 1) * P, dc * NQ:(dc + 1) * NQ],
                    in_=ot,
                )

            # ---------- emission schedule ----------
            w_dmas()
            kq_dmas("k", xkT, 0)
            xv_dmas(0)
            kq_dmas("q", xqT, 0)
            for mt in range(N_MT):
                kq_part(wk_sb, bk_sb, kTt, 1.0, "k", 0, mt)
            for tt in range(4):
                v_proj_tile(tt)
            for mt in range(N_MT):
                kq_part(wq_sb, bq_sb, qTt, scale, "q", 0, mt)

            for qc in range(N_QC):
                if qc < N_QC - 1:  # prefetch next chunk's activations
                    kq_dmas("k", xkT, qc + 1)
                    xv_dmas(qc + 1)
                    kq_dmas("q", xqT, qc + 1)
                if qc > 0:  # k/v of chunk qc feed only this wave's diag blocks
                    for mt in range(N_MT):
                        kq_part(wk_sb, bk_sb, kTt, 1.0, "k", qc, mt)
                    for tt in range(4 * qc, 4 * qc + 4):
                        v_proj_tile(tt)
                last = qc == N_QC - 1
                for mt in range(N_MT):
                    attention(mt, qc)
                    if qc > 0 and (not last or mt < 2):
                        # o_proj of the previous chunk as PE fill; in the last
                        # wave hold half of it back as tail filler below
                        o_proj_piece(qc - 1, mt, 0)
                        o_proj_piece(qc - 1, mt, 1)
                    if not last:
                        kq_part(wq_sb, bq_sb, qTt, scale, "q", qc + 1, mt)
                if last:
                    # leftover o_proj(qc-1) chains are ready immediately and
                    # keep the PE fed while the final attention's denominator
                    # chain (DMA transpose + reciprocal) completes
                    for j in (2, 3):
                        for dc in range(2):
                            o_proj_piece(qc - 1, j, dc)
                    for j in range(4):
                        for dc in range(2):
                            o_proj_piece(qc, j, dc)
    nc.finalize()
    return nc


def make_in_maps(Q, K, V, Wq, bq, Wk, bk, Wv, bv, Wo, bo, attn_mask=None):
    """Build the 8 per-core input maps from full (unsharded) inputs."""
    Q = np.asarray(Q, np.float32)
    K = np.asarray(K, np.float32)
    V = np.asarray(V, np.float32)
    Wq = np.asarray(Wq, np.float32)
    Wk = np.asarray(Wk, np.float32)
    Wv = np.asarray(Wv, np.float32)
    Wo = np.asarray(Wo, np.float32)
    bq = np.asarray(bq, np.float32)
    bk = np.asarray(bk, np.float32)
    bv = np.asarray(bv, np.float32)

    i_idx = np.arange(P)[:, None]
    j_idx = np.arange(P)[None, :]
    maskc = (i_idx <= j_idx).astype(NPBF16)

    xT = {}
    for b in range(B):
        xT[b] = tuple(
            np.ascontiguousarray(X[b].T).astype(NPBF16) for X in (Q, K, V)
        )
    grp = {}
    for g in range(TP):
        sl = slice(g * GD, (g + 1) * GD)
        grp[g] = dict(
            wqT=np.ascontiguousarray(Wq[sl, :].T).astype(NPBF16),
            wkT=np.ascontiguousarray(Wk[sl, :].T).astype(NPBF16),
            wvT=np.ascontiguousarray(Wv[sl, :].T).astype(NPBF16),
            woT=np.ascontiguousarray(Wo[:, sl].T).astype(NPBF16),
            bq=np.ascontiguousarray(bq[sl].reshape(N_MT, P).T).astype(np.float32),
            bk=np.ascontiguousarray(bk[sl].reshape(N_MT, P).T).astype(np.float32),
            bv=np.ascontiguousarray(bv[sl].reshape(1, GD)).astype(np.float32),
        )
    in_maps = []
    for c in range(2 * B):
        b, g = c // 2, c % 2
        m = dict(grp[g])
        m["xqT"], m["xkT"], m["xvT"] = xT[b]
        m["maskc"] = maskc
        in_maps.append(m)
    return in_maps


def assemble_output(results, bo):
    bo = np.asarray(bo, np.float32)
    out = np.empty((B, L, D_MODEL), np.float32)
    for b in range(B):
        out[b] = (results[2 * b]["out"].astype(np.float32)
                  + results[2 * b + 1]["out"].astype(np.float32) + bo)
    return out


_NC_CACHE = None


def kernel(**inputs) -> np.ndarray:
    global _NC_CACHE
    from concourse.bass_utils import run_bass_kernel_spmd

    if _NC_CACHE is None:
        _NC_CACHE = build_nc()
    in_maps = make_in_maps(**inputs)
    res = run_bass_kernel_spmd(_NC_CACHE, in_maps, core_ids=list(range(2 * B)))
    return assemble_output(res.results, inputs["bo"])
